# revision 14
# baseline (speedup 1.0000x reference)
import sys

if "/opt/trn_rl_repo" not in sys.path:
    sys.path.insert(0, "/opt/trn_rl_repo")

from contextlib import ExitStack

import numpy as np
import concourse.bass as bass
import concourse.mybir as mybir
from concourse.bass_utils import run_bass_kernel_spmd

# Problem: loss = sum_b ||cos(2pi(output_b-0.5))|| * ||cos(2pi(target_b-0.5))||
# for output/target of shape [4096, 4096] f32, values in [0, 1).
#
# Math used on device: with theta = 2pi*x - pi (in [-pi, pi), where the Sin
# LUT is accurate), s = sin(theta) and cos^2(2pi*(x-0.5)) = 1 - s^2. So per-row
# sumsq = N - sum(s^2). The device returns per-tile partial sum(s^2) per row;
# sqrt/product/final sum happen on host in float64.
#
# Pipeline shape: HBM-bound (16.78 MB/core at ~415 GB/s/core = ~40.5 us
# stream, ending ~49.4 us into the kernel). 2048-wide tiles keep both
# engines inside the 2.53 us/tile DMA cadence (act Sin 1.86 us, DVE
# square-reduce 2.30 us), so neither accumulates backlog mid-stream. The
# first tiles are 1024 so the first Sin starts early, and the last
# row-block tapers smoothly (1536/1152/768/384/256) so the post-stream
# land->sin->reduce serial tail is as short as possible.

B, N = 4096, 4096
N_CORES = 8
ROWS_PER_CORE = B // N_CORES  # 512
P = 128
ROW_BLOCKS = ROWS_PER_CORE // P  # 4
MAX_W = 2048
TWO_PI = 2.0 * np.pi

# (tensor_idx, row_block, col_off, width) in stream order.
_TILES = [
    (0, 0, 0, 1024),
    (0, 0, 1024, 1024),
    (0, 0, 2048, 2048),
    (0, 1, 0, 2048),
    (0, 1, 2048, 2048),
    (0, 2, 0, 2048),
    (0, 2, 2048, 2048),
    (0, 3, 0, 2048),
    (0, 3, 2048, 2048),
    (1, 0, 0, 2048),
    (1, 0, 2048, 2048),
    (1, 1, 0, 2048),
    (1, 1, 2048, 2048),
    (1, 2, 0, 2048),
    (1, 2, 2048, 2048),
    (1, 3, 0, 1536),
    (1, 3, 1536, 1152),
    (1, 3, 2688, 768),
    (1, 3, 3456, 384),
    (1, 3, 3840, 256),
]
N_TILES = len(_TILES)  # 20
N_EARLY = 15  # tiles whose acc columns are flushed before the tail
N_BUF = 8
N_RES = 8

_CACHE = {}


def _build():
    nc = bass.Bass()
    o_ext = nc.declare_dram_parameter(
        "output", [ROWS_PER_CORE, N], mybir.dt.float32, isOutput=False
    )
    t_ext = nc.declare_dram_parameter(
        "target", [ROWS_PER_CORE, N], mybir.dt.float32, isOutput=False
    )
    acc_ext = nc.declare_dram_parameter(
        "acc", [P, N_TILES], mybir.dt.float32, isOutput=True
    )

    exts = (o_ext, t_ext)
    tiles = [
        (exts[ti][rb * P : (rb + 1) * P, off : off + w], w)
        for ti, rb, off, w in _TILES
    ]

    one_ap = nc.const_aps.tensor(1.0, (P, 1), mybir.dt.float32)

    with (
        ExitStack() as ctx,
        nc.semaphore("dma_sem") as dma_sem,
        nc.semaphore("act_sem") as act_sem,
        nc.semaphore("dve_sem") as dve_sem,
        nc.Block(no_gpsimd_drain=True) as block,
    ):
        in_bufs = [
            ctx.enter_context(
                nc.sbuf_tensor(f"in_buf{i}", [P, MAX_W], mybir.dt.float32)
            )
            for i in range(N_BUF)
        ]
        # bf16 sin values: halves SBUF traffic; the f32 accumulator keeps the
        # reduced sums accurate (final rel err ~1e-4).
        res_bufs = [
            ctx.enter_context(
                nc.sbuf_tensor(f"res_buf{i}", [P, MAX_W], mybir.dt.bfloat16)
            )
            for i in range(N_RES)
        ]
        scratch = ctx.enter_context(
            nc.sbuf_tensor("scratch", [P, 1], mybir.dt.bfloat16)
        )
        bias_t = ctx.enter_context(
            nc.sbuf_tensor("bias_neg_pi", [P, 1], mybir.dt.float32)
        )
        acc = ctx.enter_context(
            nc.sbuf_tensor("acc_sb", [P, N_TILES], mybir.dt.float32)
        )

        @block.sync
        def _(sync):
            for i, (dram_ap, w) in enumerate(tiles):
                if i >= N_BUF:
                    # Sin of tile i-N_BUF must be done reading this buffer.
                    sync.wait_ge(act_sem, i - N_BUF + 1)
                sync.dma_start(
                    out=in_bufs[i % N_BUF][:, :w], in_=dram_ap
                ).then_inc(dma_sem, 16)
            # Flush the bulk of acc early so only a tiny DMA remains at the end.
            sync.wait_ge(dve_sem, N_EARLY)
            sync.dma_start(
                out=acc_ext[:, :N_EARLY], in_=acc[:, :N_EARLY]
            ).then_inc(dma_sem, 16)
            sync.wait_ge(dve_sem, N_TILES)
            sync.dma_start(
                out=acc_ext[:, N_EARLY:], in_=acc[:, N_EARLY:]
            ).then_inc(dma_sem, 16)

        @block.scalar
        def _(scalar):
            # bias_t = -pi, produced on the consuming engine (no cross-engine
            # sync needed; the pre-registered const-1.0 AP is barrier-ready).
            scalar.mul(bias_t[:], one_ap, float(-np.pi))
            for i, (_, w) in enumerate(tiles):
                scalar.wait_ge(dma_sem, 16 * (i + 1))
                if i >= N_RES:
                    # STT of tile i-N_RES must be done reading res_bufs[i%N_RES].
                    scalar.wait_ge(dve_sem, i - N_RES + 1)
                scalar.activation(
                    res_bufs[i % N_RES][:, :w],
                    in_bufs[i % N_BUF][:, :w],
                    mybir.ActivationFunctionType.Sin,
                    bias=bias_t[:],
                    scale=TWO_PI,
                ).then_inc(act_sem, 1)

        @block.vector
        def _(vector):
            for i, (_, w) in enumerate(tiles):
                vector.wait_ge(act_sem, i + 1)
                vector.scalar_tensor_tensor(
                    out=scratch[:].broadcast_to([P, w]),
                    in0=res_bufs[i % N_RES][:, :w],
                    scalar=1.0,
                    in1=res_bufs[i % N_RES][:, :w],
                    op0=mybir.AluOpType.mult,
                    op1=mybir.AluOpType.mult,
                    accum_out=acc[:, i : i + 1],
                ).then_inc(dve_sem, 1)

    return nc


def _get_nc():
    if "nc" not in _CACHE:
        _CACHE["nc"] = _build()
    return _CACHE["nc"]


def kernel(output: np.ndarray, target: np.ndarray) -> np.ndarray:
    output = np.ascontiguousarray(output, dtype=np.float32)
    target = np.ascontiguousarray(target, dtype=np.float32)
    nc = _get_nc()
    in_maps = [
        {
            "output": output[c * ROWS_PER_CORE : (c + 1) * ROWS_PER_CORE],
            "target": target[c * ROWS_PER_CORE : (c + 1) * ROWS_PER_CORE],
        }
        for c in range(N_CORES)
    ]
    results = run_bass_kernel_spmd(nc, in_maps, core_ids=list(range(N_CORES))).results

    total = 0.0
    for core in range(N_CORES):
        acc = results[core]["acc"].astype(np.float64)  # [P, N_TILES]
        # Rebuild per-(tensor, row_block) sum of sin^2 from tile columns.
        sumsq = np.zeros((2, ROW_BLOCKS, P), dtype=np.float64)
        for j, (ti, rb, off, w) in enumerate(_TILES):
            sumsq[ti, rb] += acc[:, j]
        so = np.maximum(float(N) - sumsq[0], 0.0)
        st = np.maximum(float(N) - sumsq[1], 0.0)
        total += np.sqrt(so * st).sum()
    return np.array(total, dtype=np.float32)
